# revision 4
# baseline (speedup 1.0000x reference)
"""Chamfer distance (pytorch3d defaults) on 8 Trainium2 NeuronCores.

Problem: gts_X, pred_X: [4, 8192, 3] fp32. loss = mean_b mean_n min_p d(x_bn, y_bp)
                                              + mean_b mean_p min_n d(x_bn, y_bp),
d = squared euclidean distance. gts_normals is unused (reference default path).

Sharding: 8 independent tasks = 4 batches x 2 directions, one per core.
Each core computes sum_q min_r d(Q_q, R_r) for its (Q, R) pair of 8192-point
clouds; host divides by (4*8192) and sums.

Device algorithm per core: d[q, r] = |Q_q|^2 + |R_r|^2 - 2 Q_q.R_r is computed
as ONE K=16 bf16 matmul per (128q x 512r) tile using an exact hi/lo bf16 split
(bf16 x bf16 products are exact in fp32, PSUM accumulates fp32, so precision is
~fp32: verified rel err ~1e-5 end to end). The DVE then min-reduces each PSUM
tile and the host-side mean is exact.
"""

import sys

sys.path.insert(0, "/opt/trn_rl_repo")

import numpy as np
import ml_dtypes

import concourse.bass as bass
import concourse.bacc as bacc
import concourse.mybir as mybir
from concourse.tile import TileContext
from concourse.bass_utils import run_bass_kernel_spmd

BF16 = ml_dtypes.bfloat16

B = 4
N = 8192
K = 16  # contraction rows after hi/lo split
MBLK = 128  # queries per row block (PSUM partitions)
NBLK = 512  # refs per matmul (one PSUM bank of fp32)
GROUP = 4  # matmul tiles per DVE reduce (4 PSUM banks at once)
NMB = N // MBLK  # 64 row blocks
NNB = N // NBLK  # 16 col tiles
NGRP = NNB // GROUP  # 4 reduce groups per row block

LAST_RESULTS = None  # BassKernelResults of the most recent run (for test.py)


def _build_bass():
    nc = bacc.Bacc("TRN2")
    lhs = nc.dram_tensor("lhs", [K, N], mybir.dt.bfloat16, kind="ExternalInput")
    rhs = nc.dram_tensor("rhs", [K, N], mybir.dt.bfloat16, kind="ExternalInput")
    out = nc.dram_tensor("out", [MBLK, 1], mybir.dt.float32, kind="ExternalOutput")

    with TileContext(nc) as tc:
        with (
            tc.tile_pool(name="data", bufs=1) as data_pool,
            tc.tile_pool(name="work", bufs=3) as work_pool,
            tc.tile_pool(name="ps", bufs=2, space="PSUM") as ps_pool,
        ):
            lhs_sb = data_pool.tile([K, N], mybir.dt.bfloat16)
            rhs_sb = data_pool.tile([K, N], mybir.dt.bfloat16)
            nc.sync.dma_start(lhs_sb[:], lhs.ap())
            nc.sync.dma_start(rhs_sb[:], rhs.ap())

            # per-row-block mins [128, 64]: blockmins[p, m] = min over all refs
            # for query (m*128 + p)
            blockmins = data_pool.tile([MBLK, NMB], mybir.dt.float32)

            for m in range(NMB):
                part = work_pool.tile([MBLK, NNB], mybir.dt.float32, tag="part")
                for g in range(NGRP):
                    ps = ps_pool.tile(
                        [MBLK, GROUP, NBLK], mybir.dt.float32, tag="ps"
                    )
                    for j in range(GROUP):
                        n = g * GROUP + j
                        nc.tensor.matmul(
                            ps[:, j, :],
                            lhs_sb[:, m * MBLK : (m + 1) * MBLK],
                            rhs_sb[:, n * NBLK : (n + 1) * NBLK],
                            start=True,
                            stop=True,
                        )
                    nc.vector.tensor_reduce(
                        part[:, g * GROUP : (g + 1) * GROUP],
                        ps[:],
                        axis=mybir.AxisListType.X,
                        op=mybir.AluOpType.min,
                    )
                nc.vector.tensor_reduce(
                    blockmins[:, m : m + 1],
                    part[:],
                    axis=mybir.AxisListType.X,
                    op=mybir.AluOpType.min,
                )

            # sum the 64 per-block mins per partition -> [128, 1]; the final
            # 128-way partition sum happens on host (128 floats per core).
            acc = data_pool.tile([MBLK, 1], mybir.dt.float32)
            nc.vector.tensor_reduce(
                acc[:],
                blockmins[:],
                axis=mybir.AxisListType.X,
                op=mybir.AluOpType.add,
            )
            nc.sync.dma_start(out.ap(), acc[:])
    return nc


def _split_bf16(v):
    """v (fp32) ~= hi + lo with both bf16; residual is O(2^-18 |v|)."""
    hi = v.astype(BF16)
    lo = (v - hi.astype(np.float32)).astype(BF16)
    return hi, lo


def _prep_core_inputs(Q, R):
    """Build the K=16 lhsT (queries) and rhs (refs) bf16 matrices so that
    lhsT.T @ rhs accumulated in fp32 equals |Q|^2 + |R|^2 - 2 Q.R.

    Rows: cross term -2 Q.R expands over (Qh+Ql).(Rh'+Rl') with R' = -2R,
    4 combos x 3 dims = 12 rows; |Q|^2 (hi+lo) against ones = 2 rows;
    ones against |R|^2 (hi+lo) = 2 rows.
    """
    Q = np.ascontiguousarray(np.asarray(Q, dtype=np.float32))
    R = np.ascontiguousarray(np.asarray(R, dtype=np.float32))
    Qh, Ql = _split_bf16(Q)  # [N, 3]
    Rh, Rl = _split_bf16(-2.0 * R)  # [N, 3]
    nQh, nQl = _split_bf16((Q * Q).sum(axis=1))  # [N]
    nRh, nRl = _split_bf16((R * R).sum(axis=1))  # [N]
    one = np.ones(N, dtype=BF16)

    L = np.empty([K, N], dtype=BF16)
    L[0:3] = Qh.T
    L[3:6] = Qh.T
    L[6:9] = Ql.T
    L[9:12] = Ql.T
    L[12] = nQh
    L[13] = nQl
    L[14] = one
    L[15] = one

    Rm = np.empty([K, N], dtype=BF16)
    Rm[0:3] = Rh.T
    Rm[3:6] = Rl.T
    Rm[6:9] = Rh.T
    Rm[9:12] = Rl.T
    Rm[12] = one
    Rm[13] = one
    Rm[14] = nRh
    Rm[15] = nRl
    return L, Rm


def kernel(gts_X, pred_X, gts_normals=None, **_ignored):
    global LAST_RESULTS
    gts_X = np.asarray(gts_X, dtype=np.float32)
    pred_X = np.asarray(pred_X, dtype=np.float32)
    assert gts_X.shape == (B, N, 3) and pred_X.shape == (B, N, 3)

    in_maps = []
    for b in range(B):
        for direction in (0, 1):
            if direction == 0:
                Qr, Rr = gts_X[b], pred_X[b]  # each gts point -> nearest pred
            else:
                Qr, Rr = pred_X[b], gts_X[b]  # each pred point -> nearest gts
            L, Rm = _prep_core_inputs(Qr, Rr)
            in_maps.append({"lhs": L, "rhs": Rm})

    nc = _build_bass()
    nc.finalize()
    res = run_bass_kernel_spmd(nc, in_maps, core_ids=list(range(8)))
    LAST_RESULTS = res

    total = 0.0
    for r in res.results:
        total += float(r["out"].astype(np.float64).sum())
    loss = total / (B * N)
    return np.asarray(loss, dtype=np.float32)
